# revision 11
# baseline (speedup 1.0000x reference)
"""Trainium2 Bass kernel for quantized Llama attention (fake-quant W8A8 + RoPE + GQA).

Full-input contract: kernel(**inputs) takes the complete tensors, shards them
across 8 NeuronCores internally (DP=2 over batch x TP=4 over heads), runs one
SPMD Bass/Tile kernel, and gathers/sums the partial outputs on host.

Hardcoded problem shape: B=2, S=2048, H=2048, NH=16, NKV=8, HD=128, THETA=1e4,
W_BIT=A_BIT=8.

Per-core device program (core c -> b = c//4 batch, g = c%4 head group):
  - quantize x^T and the weight shards on device (round-half-even via the
    +/-1.5*2^23 magic-add trick; scales are host-computed scalars passed in)
  - integer QKV projections in bf16 (int values <= 127 are exact in bf16),
    PSUM f32 accumulate is exact
  - RoPE applied in [d, tok] layout; rotate-half done with a +/-1 permutation
    matmul on the PE; sin/cos tables built on device from position_ids via
    Cody-Waite range reduction + ACT Sin
  - flash-style causal attention per head in S^T orientation (scores
    transposed: [k_part, q_free]) with f32r matmuls; no row-max subtraction
    (scores are bounded ~ +/-6 for this problem); softmax denominator via a
    DVE-accumulated P-sum + ones-vector matmul; normalization applied through
    a PE-broadcast reciprocal tile
  - global absmax of attn via gpsimd partition_all_reduce + an 8-core
    AllReduce(max) collective of one scalar
  - attn quantized to int-in-bf16, o_proj in bf16 against the wo shard,
    partial [S, H] written out; host sums the 4 TP partials per batch
"""

import sys
import numpy as np

try:
    import concourse  # noqa: F401
except ImportError:  # pragma: no cover
    sys.path.insert(0, "/opt/trn_rl_repo")

import concourse.bass as bass  # noqa: E402,F401
import concourse.mybir as mybir  # noqa: E402
import concourse.tile as tile  # noqa: E402
from concourse import bacc, bass_isa  # noqa: E402
from concourse.bass_utils import run_bass_kernel_spmd  # noqa: E402

F32 = mybir.dt.float32
F32R = mybir.dt.float32r
BF16 = mybir.dt.bfloat16
ALU = mybir.AluOpType
ACTF = mybir.ActivationFunctionType

B, S, H = 2, 2048, 2048
NH, NKV, HD = 16, 8, 128
THETA = 10000.0
QMAX = 127.0

DP, TP = 2, 4          # batch groups x head groups
NCORES = DP * TP
QH_LOC = NH // TP      # 4 q heads per core
KVH_LOC = NKV // TP    # 2 kv heads per core
DQ_LOC = QH_LOC * HD   # 512
DKV_LOC = KVH_LOC * HD  # 256

NHB = H // 128         # 16 hidden blocks
NTB = S // 128         # 16 token blocks
NTC = S // 512         # 4 token chunks

MAGIC = 12582912.0     # 1.5 * 2**23: (x + MAGIC) - MAGIC == round-half-even(x)
TWO_PI = 6.283185307179586
CW1 = 6.28125
_c2bits = np.float32(TWO_PI - CW1).view(np.uint32) & np.uint32(0xFFFFF000)
CW2 = float(np.uint32(_c2bits).view(np.float32))
CW3 = float(np.float32(TWO_PI - CW1 - CW2))
INV_2PI = float(np.float32(1.0 / TWO_PI))
HALF_PI = float(np.float32(np.pi / 2))


def _emit(nc, tc, xT, wqT, wkT, wvT, woT, pos, scales, rt, invf, out):
    from contextlib import ExitStack

    with ExitStack() as ctx:
        cst = ctx.enter_context(tc.tile_pool(name="cst", bufs=1))
        psum = ctx.enter_context(tc.tile_pool(name="psum", bufs=1, space="PSUM"))
        dram = ctx.enter_context(tc.tile_pool(name="dram", bufs=1, space="DRAM"))

        # ---------------- constants ----------------
        scl_row = cst.tile([1, 16], F32, tag="scl_row")
        nc.sync.dma_start(scl_row[:], scales[:])
        scl = cst.tile([128, 16], F32, tag="scl")
        nc.gpsimd.partition_broadcast(scl[:], scl_row[:], channels=128)
        inv_sx = scl[:, 0:1]
        inv_swq = scl[:, 1:2]
        inv_swk = scl[:, 2:3]
        inv_swv = scl[:, 3:4]
        inv_swo = scl[:, 4:5]
        qscale = scl[:, 5:6]
        kscale = scl[:, 6:7]
        swo = scl[:, 8:9]
        vscale_11 = scl_row[0:1, 7:8]   # [1,1] scalar for [1,512] recip tiles

        rt_f = cst.tile([HD, HD], F32, tag="rt_f")
        nc.sync.dma_start(rt_f[:], rt[:])
        rt_r = cst.tile([HD, HD], F32R, tag="rt_r")
        nc.vector.tensor_copy(rt_r[:], rt_f[:])

        invf_s = cst.tile([128, 1], F32, tag="invf_s")
        nc.sync.dma_start(invf_s[:], invf[:])
        pos_s = cst.tile([1, S], F32, tag="pos_s")
        nc.sync.dma_start(pos_s[:], pos[:])

        ones_row = cst.tile([1, 128], F32, tag="ones_row")  # partition-bcast lhsT
        nc.vector.memset(ones_row[:], 1.0)
        ones_col_f = cst.tile([128, 1], F32, tag="ones_col_f")
        nc.vector.memset(ones_col_f[:], 1.0)
        ones_col = cst.tile([128, 1], F32R, tag="ones_col")  # partition-sum lhsT
        nc.vector.tensor_copy(ones_col[:], ones_col_f[:])
        halfpi = cst.tile([128, 1], F32, tag="halfpi")
        nc.vector.memset(halfpi[:], HALF_PI)

        # causal masks for the 4 diagonal sub-blocks of a [128k x 512q] tile:
        # mask_j[kp, qf] = 1 if kp <= qf - 128*j else 0
        masks = []
        for j in range(4):
            m = cst.tile([128, 512], F32, name=f"mask{j}", tag=f"mask{j}")
            nc.gpsimd.memset(m[:], 1.0)
            nc.gpsimd.affine_select(
                out=m[:], in_=m[:], compare_op=ALU.is_ge, fill=0.0,
                base=-128 * j, pattern=[[1, 512]], channel_multiplier=-1,
            )
            masks.append(m)

        amax_acc = cst.tile([128, 1], F32, tag="amax_acc")
        nc.vector.memset(amax_acc[:], 0.0)

        def quantize_dma(src_ap, dst_bf16, inv_scale_ap, pool, shape, tagp,
                         nbufs=3):
            """dst = round_half_even(src * inv_scale) as bf16 ints."""
            f = pool.tile(shape, F32, tag=f"{tagp}_f", bufs=nbufs)
            nc.sync.dma_start(f[:], src_ap)
            t = pool.tile(shape, F32, tag=f"{tagp}_t", bufs=nbufs)
            nc.scalar.activation(t[:], f[:], ACTF.Copy,
                                 bias=MAGIC, scale=inv_scale_ap)
            nc.vector.tensor_scalar_add(dst_bf16, t[:], -MAGIC)

        # ============ persistent activations for projection+attention =======
        acts = ctx.enter_context(tc.tile_pool(name="acts", bufs=1))
        qT = [acts.tile([128, S], F32R, name=f"qT{j}", tag=f"qT{j}")
              for j in range(QH_LOC)]
        kT = [acts.tile([128, S], F32R, name=f"kT{j}", tag=f"kT{j}")
              for j in range(KVH_LOC)]
        v_sb = [acts.tile([128, DKV_LOC], F32R, name=f"v{t}", tag=f"v{t}")
                for t in range(NTB)]

        # ============ phase 1: rope tables + weights + projections ==========
        with tc.tile_pool(name="tbl", bufs=1) as tbl:
            sin_t = tbl.tile([128, S], F32, tag="sin_t")
            cos_t = tbl.tile([128, S], F32, tag="cos_t")
            with tc.tile_pool(name="ropetmp", bufs=1) as rtp:
                for c in range(NTC):
                    sl = slice(512 * c, 512 * (c + 1))
                    pbc = psum.tile([128, 512], F32, tag="psA", bufs=3,
                                    name=f"posb{c}")
                    nc.tensor.matmul(pbc[:], ones_row[:], pos_s[0:1, sl],
                                     start=True, stop=True)
                    emb = rtp.tile([128, 512], F32, tag="emb", bufs=2)
                    nc.vector.tensor_scalar_mul(emb[:], pbc[:], invf_s[:, 0:1])
                    k1 = rtp.tile([128, 512], F32, tag="k1", bufs=2)
                    nc.scalar.activation(k1[:], emb[:], ACTF.Copy,
                                         bias=MAGIC, scale=INV_2PI)
                    nc.vector.tensor_scalar_add(k1[:], k1[:], -MAGIC)
                    red = rtp.tile([128, 512], F32, tag="red", bufs=2)
                    nc.vector.cody_waite_cascade(red[:], emb[:], k1[:],
                                                 CW1, CW2, CW3)
                    nc.scalar.activation(sin_t[:, sl], red[:], ACTF.Sin)
                    k2 = rtp.tile([128, 512], F32, tag="k2", bufs=2)
                    nc.scalar.activation(k2[:], emb[:], ACTF.Copy,
                                         bias=0.25, scale=INV_2PI)
                    nc.vector.tensor_scalar_add(k2[:], k2[:], MAGIC)
                    nc.vector.tensor_scalar_add(k2[:], k2[:], -MAGIC)
                    red2 = rtp.tile([128, 512], F32, tag="red2", bufs=2)
                    nc.vector.cody_waite_cascade(red2[:], emb[:], k2[:],
                                                 CW1, CW2, CW3)
                    nc.scalar.activation(cos_t[:, sl], red2[:], ACTF.Sin,
                                         bias=halfpi[:, 0:1])

            with tc.tile_pool(name="wqkv", bufs=1) as wqkv:
                wq_q, wk_q, wv_q = [], [], []
                with tc.tile_pool(name="wtmp", bufs=1) as wtp:
                    for h in range(NHB):
                        wq_b = wqkv.tile([128, DQ_LOC], BF16, tag=f"wq{h}")
                        quantize_dma(wqT[128 * h:128 * (h + 1), :], wq_b[:],
                                     inv_swq, wtp, [128, DQ_LOC], "wq")
                        wq_q.append(wq_b)
                    for h in range(NHB):
                        wk_b = wqkv.tile([128, DKV_LOC], BF16, tag=f"wk{h}")
                        quantize_dma(wkT[128 * h:128 * (h + 1), :], wk_b[:],
                                     inv_swk, wtp, [128, DKV_LOC], "wk")
                        wk_q.append(wk_b)
                    for h in range(NHB):
                        wv_b = wqkv.tile([128, DKV_LOC], BF16, tag=f"wv{h}")
                        quantize_dma(wvT[128 * h:128 * (h + 1), :], wv_b[:],
                                     inv_swv, wtp, [128, DKV_LOC], "wv")
                        wv_q.append(wv_b)

                def rope(dst_slice, ps_proj, scale_ap, prj, tc_idx):
                    sl = slice(512 * tc_idx, 512 * (tc_idx + 1))
                    qs = prj.tile([128, 512], F32R, tag="qs", bufs=3)
                    nc.scalar.activation(qs[:], ps_proj, ACTF.Copy,
                                         scale=scale_ap)
                    rot = psum.tile([128, 512], F32, tag="psB", bufs=2,
                                    name="rot")
                    nc.tensor.matmul(rot[:], rt_r[:], qs[:],
                                     start=True, stop=True)
                    t1 = prj.tile([128, 512], F32, tag="t1", bufs=2)
                    nc.vector.tensor_tensor(t1[:], qs[:], cos_t[:, sl],
                                            ALU.mult)
                    t2 = prj.tile([128, 512], F32, tag="t2", bufs=2)
                    nc.vector.tensor_tensor(t2[:], rot[:], sin_t[:, sl],
                                            ALU.mult)
                    nc.vector.tensor_tensor(dst_slice, t1[:], t2[:], ALU.add)

                with tc.tile_pool(name="prj", bufs=1) as prj:
                    for tci in range(NTC):
                        tsl = slice(512 * tci, 512 * (tci + 1))
                        xq = []
                        for h in range(NHB):
                            xq_b = prj.tile([128, 512], BF16, tag=f"xq{h}",
                                            bufs=2)
                            quantize_dma(xT[128 * h:128 * (h + 1), tsl],
                                         xq_b[:], inv_sx, prj,
                                         [128, 512], "x")
                            xq.append(xq_b)
                        for j in range(QH_LOC):
                            ps = psum.tile([128, 512], F32, tag="psA", bufs=3,
                                           name=f"q{j}_{tci}")
                            for h in range(NHB):
                                nc.tensor.matmul(
                                    ps[:], wq_q[h][:, 128 * j:128 * (j + 1)],
                                    xq[h][:],
                                    start=(h == 0), stop=(h == NHB - 1))
                            rope(qT[j][:, tsl], ps[:], qscale, prj, tci)
                        for j in range(KVH_LOC):
                            ps = psum.tile([128, 512], F32, tag="psA", bufs=3,
                                           name=f"k{j}_{tci}")
                            for h in range(NHB):
                                nc.tensor.matmul(
                                    ps[:], wk_q[h][:, 128 * j:128 * (j + 1)],
                                    xq[h][:],
                                    start=(h == 0), stop=(h == NHB - 1))
                            rope(kT[j][:, tsl], ps[:], kscale, prj, tci)
                        for tb in range(4):
                            t_glob = 4 * tci + tb
                            ps = psum.tile([128, DKV_LOC], F32, tag="psA",
                                           bufs=3, name=f"v{t_glob}")
                            for h in range(NHB):
                                nc.tensor.matmul(
                                    ps[:], xq[h][:, 128 * tb:128 * (tb + 1)],
                                    wv_q[h][:],
                                    start=(h == 0), stop=(h == NHB - 1))
                            nc.scalar.activation(v_sb[t_glob][:], ps[:],
                                                 ACTF.Copy)

        # ============ phase 2: attention ====================================
        aqp = ctx.enter_context(tc.tile_pool(name="aqp", bufs=1))
        with tc.tile_pool(name="attnp", bufs=1) as attnp:
            attnT = [attnp.tile([128, S], F32, name=f"attnT{j}",
                                tag=f"attnT{j}") for j in range(QH_LOC)]
            with tc.tile_pool(name="att", bufs=1) as att:
                for j in range(QH_LOC):
                    kv = j // 2
                    vcol = slice(128 * kv, 128 * kv + 128)
                    for qc in range(NTC):
                        qsl = slice(512 * qc, 512 * (qc + 1))
                        nkb = 4 * (qc + 1)       # causal k blocks 0..4qc+3
                        aps = psum.tile([128, 512], F32, tag="psB", bufs=2,
                                        name=f"a{j}_{qc}")
                        pacc = att.tile([128, 512], F32, tag="pacc", bufs=2)
                        for kb in range(nkb):
                            sps = psum.tile([128, 512], F32, tag="psA",
                                            bufs=3, name=f"s{j}_{qc}_{kb}")
                            nc.tensor.matmul(
                                sps[:], kT[kv][:, 128 * kb:128 * (kb + 1)],
                                qT[j][:, qsl], start=True, stop=True)
                            pt = att.tile([128, 512], F32R, tag="pt", bufs=3)
                            nc.scalar.activation(pt[:], sps[:], ACTF.Exp)
                            if kb >= 4 * qc:  # diagonal block: causal mask
                                nc.vector.tensor_tensor(
                                    pt[:], pt[:], masks[kb - 4 * qc][:],
                                    ALU.mult)
                            if kb == 0:
                                nc.vector.tensor_copy(pacc[:], pt[:])
                            else:
                                nc.vector.tensor_tensor(pacc[:], pacc[:],
                                                        pt[:], ALU.add)
                            nc.tensor.matmul(aps[:], v_sb[kb][:, vcol], pt[:],
                                             start=(kb == 0),
                                             stop=(kb == nkb - 1))
                        sums = psum.tile([1, 512], F32, tag="psS", bufs=2,
                                         name=f"sm{j}_{qc}")
                        nc.tensor.matmul(sums[:], ones_col_f[:], pacc[:],
                                         start=True, stop=True)
                        sums_sb = att.tile([1, 512], F32, tag="sums_sb",
                                           bufs=2)
                        nc.vector.tensor_copy(sums_sb[:], sums[:])
                        rec = att.tile([1, 512], F32, tag="rec", bufs=2)
                        scr = att.tile([1, 512], F32, tag="scr", bufs=2)
                        nc.vector.reciprocal_approx_accurate(rec[:],
                                                             sums_sb[:],
                                                             scr[:])
                        nc.vector.tensor_scalar_mul(rec[:], rec[:], vscale_11)
                        rb = psum.tile([128, 512], F32, tag="psS", bufs=2,
                                       name=f"rb{j}_{qc}")
                        nc.tensor.matmul(rb[:], ones_row[:], rec[:],
                                         start=True, stop=True)
                        rb_sb = att.tile([128, 512], F32, tag="rb_sb", bufs=2)
                        nc.scalar.activation(rb_sb[:], rb[:], ACTF.Copy)
                        nc.vector.tensor_tensor(attnT[j][:, qsl], aps[:],
                                                rb_sb[:], ALU.mult)
                        mx = att.tile([128, 1], F32, tag="mx", bufs=2)
                        nc.vector.tensor_reduce(mx[:], attnT[j][:, qsl],
                                                axis=mybir.AxisListType.X,
                                                op=ALU.max,
                                                apply_absolute_value=True)
                        nc.vector.tensor_tensor(amax_acc[:], amax_acc[:],
                                                mx[:], ALU.max)

            # ---------------- global amax collective ----------------
            amax_red = cst.tile([128, 1], F32, tag="amax_red")
            nc.gpsimd.partition_all_reduce(amax_red[:], amax_acc[:],
                                           channels=128,
                                           reduce_op=bass_isa.ReduceOp.max)
            pad = cst.tile([1, 8], F32, tag="pad")
            nc.vector.memset(pad[:], 0.0)
            nc.vector.tensor_copy(pad[0:1, 0:1], amax_red[0:1, 0:1])
            cc_in = dram.tile([1, 8], F32, name="cc_in", tag="cc_in")
            cc_out = dram.tile([1, 8], F32, name="cc_out", tag="cc_out",
                               addr_space="Shared")
            nc.sync.dma_start(cc_in[:], pad[:])
            nc.gpsimd.collective_compute(
                "AllReduce", ALU.max,
                replica_groups=[list(range(NCORES))],
                ins=[cc_in.opt()], outs=[cc_out.opt()],
            )
            gmax_row = cst.tile([1, 8], F32, tag="gmax_row")
            nc.sync.dma_start(gmax_row[:], cc_out[:])
            gmax = cst.tile([128, 8], F32, tag="gmax")
            nc.gpsimd.partition_broadcast(gmax[:], gmax_row[:], channels=128)
            sa = cst.tile([128, 1], F32, tag="sa")
            nc.vector.tensor_scalar(out=sa[:], in0=gmax[:, 0:1],
                                    scalar1=1.0 / QMAX, scalar2=1e-8,
                                    op0=ALU.mult, op1=ALU.max)
            inv_sa = cst.tile([128, 1], F32, tag="inv_sa")
            nc.vector.reciprocal(inv_sa[:], sa[:])
            osc = cst.tile([128, 1], F32, tag="osc")
            nc.vector.tensor_tensor(osc[:], sa[:], swo, ALU.mult)

            # ---------------- attn quantization ----------------
            aq = [aqp.tile([128, S], BF16, name=f"aq{j}", tag=f"aq{j}")
                  for j in range(QH_LOC)]
            with tc.tile_pool(name="qtz", bufs=1) as qtz:
                for j in range(QH_LOC):
                    t = qtz.tile([128, S], F32, tag="aqt", bufs=2)
                    nc.scalar.activation(t[:], attnT[j][:], ACTF.Copy,
                                         bias=MAGIC, scale=inv_sa[:, 0:1])
                    nc.vector.tensor_scalar_add(aq[j][:], t[:], -MAGIC)

        # ============ phase 3: o_proj =======================================
        with tc.tile_pool(name="wop", bufs=1) as wop:
            wo_q = []
            with tc.tile_pool(name="wotmp", bufs=1) as wtp2:
                for dj in range(DQ_LOC // 128):
                    wo_b = wop.tile([128, H], BF16, tag=f"wo{dj}")
                    quantize_dma(woT[128 * dj:128 * (dj + 1), :], wo_b[:],
                                 inv_swo, wtp2, [128, H], "wo", nbufs=2)
                    wo_q.append(wo_b)

            with tc.tile_pool(name="opj", bufs=1) as opj:
                for tb in range(NTB):
                    for hc in range(H // 512):
                        ops = psum.tile([128, 512], F32, tag="psA", bufs=3,
                                        name=f"o{tb}_{hc}")
                        for dj in range(DQ_LOC // 128):
                            nc.tensor.matmul(
                                ops[:], aq[dj][:, 128 * tb:128 * (tb + 1)],
                                wo_q[dj][:, 512 * hc:512 * (hc + 1)],
                                start=(dj == 0),
                                stop=(dj == DQ_LOC // 128 - 1))
                        og = opj.tile([128, 512], F32, tag="og", bufs=3)
                        nc.scalar.activation(og[:], ops[:], ACTF.Copy,
                                             scale=osc[:, 0:1])
                        nc.sync.dma_start(
                            out[128 * tb:128 * (tb + 1),
                                512 * hc:512 * (hc + 1)],
                            og[:])


def _build():
    nc = bacc.Bacc("TRN2", target_bir_lowering=False, debug=False,
                   num_devices=NCORES)
    xT = nc.dram_tensor("xT", [H, S], F32, kind="ExternalInput")
    wqT = nc.dram_tensor("wqT", [H, DQ_LOC], F32, kind="ExternalInput")
    wkT = nc.dram_tensor("wkT", [H, DKV_LOC], F32, kind="ExternalInput")
    wvT = nc.dram_tensor("wvT", [H, DKV_LOC], F32, kind="ExternalInput")
    woT = nc.dram_tensor("woT", [DQ_LOC, H], F32, kind="ExternalInput")
    pos = nc.dram_tensor("pos", [1, S], F32, kind="ExternalInput")
    scales = nc.dram_tensor("scales", [1, 16], F32, kind="ExternalInput")
    rt = nc.dram_tensor("rt", [HD, HD], F32, kind="ExternalInput")
    invf = nc.dram_tensor("invf", [128, 1], F32, kind="ExternalInput")
    out = nc.dram_tensor("out", [S, H], F32, kind="ExternalOutput")

    with tile.TileContext(nc) as tc:
        _emit(nc, tc, xT[:], wqT[:], wkT[:], wvT[:], woT[:], pos[:],
              scales[:], rt[:], invf[:], out[:])
    nc.compile()
    return nc


_CACHED = {}
_RUN_KWARGS = {}   # test harness can set {"trace": True, ...}
_LAST = {}         # last BassKernelResults (for profiling in test harness)


def _get_nc():
    if "nc" not in _CACHED:
        _CACHED["nc"] = _build()
    return _CACHED["nc"]


def _host_scales(hidden_states, wq, wk, wv, wo):
    def fq_scale(t):
        return max(float(np.abs(t).max()) / QMAX, 1e-8)

    sx = fq_scale(hidden_states)
    swq = fq_scale(wq)
    swk = fq_scale(wk)
    swv = fq_scale(wv)
    swo = fq_scale(wo)
    s = np.zeros((1, 16), np.float32)
    s[0, 0] = 1.0 / sx
    s[0, 1] = 1.0 / swq
    s[0, 2] = 1.0 / swk
    s[0, 3] = 1.0 / swv
    s[0, 4] = 1.0 / swo
    s[0, 5] = np.float32(sx) * np.float32(swq) / np.float32(np.sqrt(HD))
    s[0, 6] = np.float32(sx) * np.float32(swk)
    s[0, 7] = np.float32(sx) * np.float32(swv)
    s[0, 8] = swo
    return s


def _invfreq():
    # match reference: inv_freq = 1/(theta ** (arange(0,HD,2,f32)/HD)), f32 ops
    e = np.arange(0, HD, 2, dtype=np.float32) / np.float32(HD)
    base = np.float32(THETA) ** e.astype(np.float32)
    invf = (np.float32(1.0) / base.astype(np.float32)).astype(np.float32)
    full = np.concatenate([invf, invf])  # emb = concat([freqs, freqs])
    return np.ascontiguousarray(full.reshape(HD, 1))


def _rot_matrix_T():
    rtm = np.zeros((HD, HD), np.float32)
    half = HD // 2
    idx = np.arange(half)
    rtm[idx, idx + half] = 1.0   # rot[m] = -q[m+64] for m < 64
    rtm[idx + half, idx] = -1.0  # rot[m] = +q[m-64] for m >= 64
    return rtm


def kernel(hidden_states, wq, wk, wv, wo, position_ids):
    hidden_states = np.asarray(hidden_states, dtype=np.float32)
    wq = np.asarray(wq, dtype=np.float32)
    wk = np.asarray(wk, dtype=np.float32)
    wv = np.asarray(wv, dtype=np.float32)
    wo = np.asarray(wo, dtype=np.float32)
    position_ids = np.asarray(position_ids)

    scales = _host_scales(hidden_states, wq, wk, wv, wo)
    invf = _invfreq()
    rtm = _rot_matrix_T()

    in_maps = []
    for c in range(NCORES):
        b, g = c // TP, c % TP
        qsl = slice(DQ_LOC * g, DQ_LOC * (g + 1))
        ksl = slice(DKV_LOC * g, DKV_LOC * (g + 1))
        in_maps.append({
            "xT": np.ascontiguousarray(hidden_states[b].T),
            "wqT": np.ascontiguousarray(wq[qsl, :].T),
            "wkT": np.ascontiguousarray(wk[ksl, :].T),
            "wvT": np.ascontiguousarray(wv[ksl, :].T),
            "woT": np.ascontiguousarray(wo[:, qsl].T),
            "pos": position_ids[b].astype(np.float32).reshape(1, S),
            "scales": scales,
            "rt": rtm,
            "invf": invf,
        })

    nc = _get_nc()
    res_obj = run_bass_kernel_spmd(nc, in_maps, list(range(NCORES)),
                                   **_RUN_KWARGS)
    _LAST["res"] = res_obj
    res = res_obj.results

    outp = np.zeros((B, S, H), np.float64)
    for c in range(NCORES):
        outp[c // TP] += res[c]["out"].astype(np.float64)
    return outp.astype(np.float32)


if __name__ == "__main__":
    rng = np.random.default_rng(0)
    ins = {
        "hidden_states": rng.standard_normal((B, S, H)).astype(np.float32),
        "wq": (rng.standard_normal((NH * HD, H)) * 0.02).astype(np.float32),
        "wk": (rng.standard_normal((NKV * HD, H)) * 0.02).astype(np.float32),
        "wv": (rng.standard_normal((NKV * HD, H)) * 0.02).astype(np.float32),
        "wo": (rng.standard_normal((H, NH * HD)) * 0.02).astype(np.float32),
        "position_ids": np.broadcast_to(np.arange(S), (B, S)).astype(np.int64),
    }
    o = kernel(**ins)
    print("out", o.shape, o.dtype, float(np.abs(o).max()))


# revision 13
# speedup vs baseline: 1.1182x; 1.1182x over previous
"""Trainium2 Bass kernel for quantized Llama attention (fake-quant W8A8 + RoPE + GQA).

Full-input contract: kernel(**inputs) takes the complete tensors, shards them
across 8 NeuronCores internally (DP=2 over batch x TP=4 over heads), runs one
SPMD Bass/Tile kernel, and gathers/sums the partial outputs on host.

Hardcoded problem shape: B=2, S=2048, H=2048, NH=16, NKV=8, HD=128, THETA=1e4,
W_BIT=A_BIT=8.

Per-core device program (core c -> b = c//4 batch, g = c%4 head group):
  - quantize x^T and the weight shards on device (round-half-even via the
    +/-1.5*2^23 magic-add trick; scales are host-computed scalars passed in)
  - integer QKV projections in bf16 (int values <= 127 are exact in bf16),
    PSUM f32 accumulate is exact
  - RoPE applied in [d, tok] layout; rotate-half done with a +/-1 permutation
    matmul on the PE; sin/cos tables built on device from position_ids via
    Cody-Waite range reduction + ACT Sin
  - flash-style causal attention per head in S^T orientation (scores
    transposed: [k_part, q_free]) with f32r matmuls; no row-max subtraction
    (scores are bounded ~ +/-6 for this problem); softmax denominator via a
    DVE-accumulated P-sum + ones-vector matmul; normalization applied through
    a PE-broadcast reciprocal tile
  - global absmax of attn via gpsimd partition_all_reduce + an 8-core
    AllReduce(max) collective of one scalar
  - attn quantized to int-in-bf16, o_proj in bf16 against the wo shard,
    partial [S, H] written out; host sums the 4 TP partials per batch
"""

import sys
import numpy as np

try:
    import concourse  # noqa: F401
except ImportError:  # pragma: no cover
    sys.path.insert(0, "/opt/trn_rl_repo")

import concourse.bass as bass  # noqa: E402,F401
import concourse.mybir as mybir  # noqa: E402
import concourse.tile as tile  # noqa: E402
from concourse import bacc, bass_isa  # noqa: E402
from concourse.bass_utils import run_bass_kernel_spmd  # noqa: E402

F32 = mybir.dt.float32
F32R = mybir.dt.float32r
BF16 = mybir.dt.bfloat16
ALU = mybir.AluOpType
ACTF = mybir.ActivationFunctionType

B, S, H = 2, 2048, 2048
NH, NKV, HD = 16, 8, 128
THETA = 10000.0
QMAX = 127.0

DP, TP = 2, 4          # batch groups x head groups
NCORES = DP * TP
QH_LOC = NH // TP      # 4 q heads per core
KVH_LOC = NKV // TP    # 2 kv heads per core
DQ_LOC = QH_LOC * HD   # 512
DKV_LOC = KVH_LOC * HD  # 256

NHB = H // 128         # 16 hidden blocks
NTB = S // 128         # 16 token blocks
NTC = S // 512         # 4 token chunks

MAGIC = 12582912.0     # 1.5 * 2**23: (x + MAGIC) - MAGIC == round-half-even(x)
TWO_PI = 6.283185307179586
CW1 = 6.28125
_c2bits = np.float32(TWO_PI - CW1).view(np.uint32) & np.uint32(0xFFFFF000)
CW2 = float(np.uint32(_c2bits).view(np.float32))
CW3 = float(np.float32(TWO_PI - CW1 - CW2))
INV_2PI = float(np.float32(1.0 / TWO_PI))
HALF_PI = float(np.float32(np.pi / 2))


def _emit(nc, tc, xT, wqT, wkT, wvT, woT, pos, scales, rt, invf, out):
    from contextlib import ExitStack

    with ExitStack() as ctx:
        cst = ctx.enter_context(tc.tile_pool(name="cst", bufs=1))
        psum = ctx.enter_context(tc.tile_pool(name="psum", bufs=1, space="PSUM"))
        dram = ctx.enter_context(tc.tile_pool(name="dram", bufs=1, space="DRAM"))

        # ---------------- constants ----------------
        scl_row = cst.tile([1, 16], F32, tag="scl_row")
        nc.sync.dma_start(scl_row[:], scales[:])
        scl = cst.tile([128, 16], F32, tag="scl")
        nc.gpsimd.partition_broadcast(scl[:], scl_row[:], channels=128)
        inv_sx = scl[:, 0:1]
        inv_swq = scl[:, 1:2]
        inv_swk = scl[:, 2:3]
        inv_swv = scl[:, 3:4]
        inv_swo = scl[:, 4:5]
        qscale = scl[:, 5:6]
        kscale = scl[:, 6:7]
        swo = scl[:, 8:9]
        vscale_11 = scl_row[0:1, 7:8]   # [1,1] scalar for [1,512] recip tiles

        rt_f = cst.tile([HD, HD], F32, tag="rt_f")
        nc.sync.dma_start(rt_f[:], rt[:])
        rt_r = cst.tile([HD, HD], F32R, tag="rt_r")
        nc.vector.tensor_copy(rt_r[:], rt_f[:])

        invf_s = cst.tile([128, 1], F32, tag="invf_s")
        nc.sync.dma_start(invf_s[:], invf[:])
        pos_s = cst.tile([1, S], F32, tag="pos_s")
        nc.sync.dma_start(pos_s[:], pos[:])

        ones_row = cst.tile([1, 128], F32, tag="ones_row")  # partition-bcast lhsT
        nc.vector.memset(ones_row[:], 1.0)
        ones_col_f = cst.tile([128, 1], F32, tag="ones_col_f")
        nc.vector.memset(ones_col_f[:], 1.0)
        ones_col = cst.tile([128, 1], F32R, tag="ones_col")  # partition-sum lhsT
        nc.vector.tensor_copy(ones_col[:], ones_col_f[:])
        ones_row_r = cst.tile([1, 128], F32R, tag="ones_row_r")
        nc.vector.tensor_copy(ones_row_r[:], ones_row[:])
        halfpi = cst.tile([128, 1], F32, tag="halfpi")
        nc.vector.memset(halfpi[:], HALF_PI)

        # causal masks for the 4 diagonal sub-blocks of a [128k x 512q] tile:
        # mask_j[kp, qf] = 1 if kp <= qf - 128*j else 0
        masks = []
        for j in range(4):
            m = cst.tile([128, 512], F32, name=f"mask{j}", tag=f"mask{j}")
            nc.gpsimd.memset(m[:], 1.0)
            nc.gpsimd.affine_select(
                out=m[:], in_=m[:], compare_op=ALU.is_ge, fill=0.0,
                base=-128 * j, pattern=[[1, 512]], channel_multiplier=-1,
            )
            masks.append(m)

        amax_acc = cst.tile([128, 1], F32, tag="amax_acc")
        nc.vector.memset(amax_acc[:], 0.0)

        def quantize_dma(src_ap, dst_bf16, inv_scale_ap, pool, shape, tagp,
                         nbufs=3):
            """dst = round_half_even(src * inv_scale) as bf16 ints."""
            f = pool.tile(shape, F32, tag=f"{tagp}_f", bufs=nbufs)
            nc.sync.dma_start(f[:], src_ap)
            t = pool.tile(shape, F32, tag=f"{tagp}_t", bufs=nbufs)
            nc.scalar.activation(t[:], f[:], ACTF.Copy,
                                 bias=MAGIC, scale=inv_scale_ap)
            nc.vector.tensor_scalar_add(dst_bf16, t[:], -MAGIC)

        # ============ persistent activations for projection+attention =======
        acts = ctx.enter_context(tc.tile_pool(name="acts", bufs=1))
        qT = [acts.tile([128, S], F32R, name=f"qT{j}", tag=f"qT{j}")
              for j in range(QH_LOC)]
        kT = [acts.tile([128, S], F32R, name=f"kT{j}", tag=f"kT{j}")
              for j in range(KVH_LOC)]
        v_sb = [acts.tile([128, DKV_LOC], F32R, name=f"v{t}", tag=f"v{t}")
                for t in range(NTB)]

        # ============ phase 1: rope tables + weights + projections ==========
        with tc.tile_pool(name="tbl", bufs=1) as tbl:
            sin_t = tbl.tile([128, S], F32, tag="sin_t")
            cos_t = tbl.tile([128, S], F32, tag="cos_t")
            with tc.tile_pool(name="ropetmp", bufs=1) as rtp:
                for c in range(NTC):
                    sl = slice(512 * c, 512 * (c + 1))
                    pbc = psum.tile([128, 512], F32, tag="psA", bufs=3,
                                    name=f"posb{c}")
                    nc.tensor.matmul(pbc[:], ones_row[:], pos_s[0:1, sl],
                                     start=True, stop=True)
                    emb = rtp.tile([128, 512], F32, tag="emb", bufs=2)
                    nc.vector.tensor_scalar_mul(emb[:], pbc[:], invf_s[:, 0:1])
                    k1 = rtp.tile([128, 512], F32, tag="k1", bufs=2)
                    nc.scalar.activation(k1[:], emb[:], ACTF.Copy,
                                         bias=MAGIC, scale=INV_2PI)
                    nc.vector.tensor_scalar_add(k1[:], k1[:], -MAGIC)
                    red = rtp.tile([128, 512], F32, tag="red", bufs=2)
                    nc.vector.cody_waite_cascade(red[:], emb[:], k1[:],
                                                 CW1, CW2, CW3)
                    nc.scalar.activation(sin_t[:, sl], red[:], ACTF.Sin)
                    k2 = rtp.tile([128, 512], F32, tag="k2", bufs=2)
                    nc.scalar.activation(k2[:], emb[:], ACTF.Copy,
                                         bias=0.25, scale=INV_2PI)
                    nc.vector.tensor_scalar_add(k2[:], k2[:], MAGIC)
                    nc.vector.tensor_scalar_add(k2[:], k2[:], -MAGIC)
                    red2 = rtp.tile([128, 512], F32, tag="red2", bufs=2)
                    nc.vector.cody_waite_cascade(red2[:], emb[:], k2[:],
                                                 CW1, CW2, CW3)
                    nc.scalar.activation(cos_t[:, sl], red2[:], ACTF.Sin,
                                         bias=halfpi[:, 0:1])

            with tc.tile_pool(name="wqkv", bufs=1) as wqkv:
                wq_q, wk_q, wv_q = [], [], []
                with tc.tile_pool(name="wtmp", bufs=1) as wtp:
                    for h in range(NHB):
                        wq_b = wqkv.tile([128, DQ_LOC], BF16, tag=f"wq{h}")
                        quantize_dma(wqT[128 * h:128 * (h + 1), :], wq_b[:],
                                     inv_swq, wtp, [128, DQ_LOC], "wq")
                        wq_q.append(wq_b)
                    for h in range(NHB):
                        wk_b = wqkv.tile([128, DKV_LOC], BF16, tag=f"wk{h}")
                        quantize_dma(wkT[128 * h:128 * (h + 1), :], wk_b[:],
                                     inv_swk, wtp, [128, DKV_LOC], "wk")
                        wk_q.append(wk_b)
                    for h in range(NHB):
                        wv_b = wqkv.tile([128, DKV_LOC], BF16, tag=f"wv{h}")
                        quantize_dma(wvT[128 * h:128 * (h + 1), :], wv_b[:],
                                     inv_swv, wtp, [128, DKV_LOC], "wv")
                        wv_q.append(wv_b)

                def rope(dst_slice, ps_proj, scale_ap, prj, tc_idx):
                    sl = slice(512 * tc_idx, 512 * (tc_idx + 1))
                    qs = prj.tile([128, 512], F32R, tag="qs", bufs=3)
                    nc.scalar.activation(qs[:], ps_proj, ACTF.Copy,
                                         scale=scale_ap)
                    rot = psum.tile([128, 512], F32, tag="psB", bufs=2,
                                    name="rot")
                    nc.tensor.matmul(rot[:], rt_r[:], qs[:],
                                     start=True, stop=True)
                    t1 = prj.tile([128, 512], F32, tag="t1", bufs=2)
                    nc.vector.tensor_tensor(t1[:], qs[:], cos_t[:, sl],
                                            ALU.mult)
                    t2 = prj.tile([128, 512], F32, tag="t2", bufs=2)
                    nc.vector.tensor_tensor(t2[:], rot[:], sin_t[:, sl],
                                            ALU.mult)
                    nc.vector.tensor_tensor(dst_slice, t1[:], t2[:], ALU.add)

                with tc.tile_pool(name="prj", bufs=1) as prj:
                    for tci in range(NTC):
                        tsl = slice(512 * tci, 512 * (tci + 1))
                        xq = []
                        for h in range(NHB):
                            xq_b = prj.tile([128, 512], BF16, tag=f"xq{h}",
                                            bufs=2)
                            quantize_dma(xT[128 * h:128 * (h + 1), tsl],
                                         xq_b[:], inv_sx, prj,
                                         [128, 512], "x")
                            xq.append(xq_b)
                        for j in range(QH_LOC):
                            ps = psum.tile([128, 512], F32, tag="psA", bufs=3,
                                           name=f"q{j}_{tci}")
                            for h in range(NHB):
                                nc.tensor.matmul(
                                    ps[:], wq_q[h][:, 128 * j:128 * (j + 1)],
                                    xq[h][:],
                                    start=(h == 0), stop=(h == NHB - 1))
                            rope(qT[j][:, tsl], ps[:], qscale, prj, tci)
                        for j in range(KVH_LOC):
                            ps = psum.tile([128, 512], F32, tag="psA", bufs=3,
                                           name=f"k{j}_{tci}")
                            for h in range(NHB):
                                nc.tensor.matmul(
                                    ps[:], wk_q[h][:, 128 * j:128 * (j + 1)],
                                    xq[h][:],
                                    start=(h == 0), stop=(h == NHB - 1))
                            rope(kT[j][:, tsl], ps[:], kscale, prj, tci)
                        for tb in range(4):
                            t_glob = 4 * tci + tb
                            ps = psum.tile([128, DKV_LOC], F32, tag="psA",
                                           bufs=3, name=f"v{t_glob}")
                            for h in range(NHB):
                                nc.tensor.matmul(
                                    ps[:], xq[h][:, 128 * tb:128 * (tb + 1)],
                                    wv_q[h][:],
                                    start=(h == 0), stop=(h == NHB - 1))
                            nc.scalar.activation(v_sb[t_glob][:], ps[:],
                                                 ACTF.Copy)

        # ============ phase 2: attention ====================================
        aqp = ctx.enter_context(tc.tile_pool(name="aqp", bufs=1))
        wop = ctx.enter_context(tc.tile_pool(name="wop", bufs=1))
        wo_q = []
        with tc.tile_pool(name="wotmp", bufs=1) as wtp2:
            for dj in range(DQ_LOC // 128):
                wo_b = wop.tile([128, H], BF16, tag=f"wo{dj}")
                for hcq in range(H // 512):
                    quantize_dma(
                        woT[128 * dj:128 * (dj + 1), 512 * hcq:512 * (hcq + 1)],
                        wo_b[:, 512 * hcq:512 * (hcq + 1)],
                        inv_swo, wtp2, [128, 512], "wo", nbufs=3)
                wo_q.append(wo_b)
        with tc.tile_pool(name="attnp", bufs=1) as attnp:
            attnT = [attnp.tile([128, S], F32, name=f"attnT{j}",
                                tag=f"attnT{j}") for j in range(QH_LOC)]
            with tc.tile_pool(name="att", bufs=1) as att:
                for j in range(QH_LOC):
                    kv = j // 2
                    vcol = slice(128 * kv, 128 * kv + 128)
                    for qc in range(NTC):
                        qsl = slice(512 * qc, 512 * (qc + 1))
                        nkb = 4 * (qc + 1)       # causal k blocks 0..4qc+3
                        aps = psum.tile([128, 512], F32, tag="psB", bufs=2,
                                        name=f"a{j}_{qc}")
                        sums = psum.tile([1, 512], F32, tag="psS", bufs=2,
                                         name=f"sm{j}_{qc}")
                        for kb in range(nkb):
                            sps = psum.tile([128, 512], F32, tag="psA",
                                            bufs=3, name=f"s{j}_{qc}_{kb}")
                            nc.tensor.matmul(
                                sps[:], kT[kv][:, 128 * kb:128 * (kb + 1)],
                                qT[j][:, qsl], start=True, stop=True)
                            pt = att.tile([128, 512], F32R, tag="pt", bufs=3)
                            nc.scalar.activation(pt[:], sps[:], ACTF.Exp)
                            if kb >= 4 * qc:  # diagonal block: causal mask
                                nc.vector.tensor_tensor(
                                    pt[:], pt[:], masks[kb - 4 * qc][:],
                                    ALU.mult)
                            nc.tensor.matmul(aps[:], v_sb[kb][:, vcol], pt[:],
                                             start=(kb == 0),
                                             stop=(kb == nkb - 1))
                            nc.tensor.matmul(sums[:], ones_col[:], pt[:],
                                             start=(kb == 0),
                                             stop=(kb == nkb - 1))
                        sums_sb = att.tile([1, 512], F32, tag="sums_sb",
                                           bufs=2)
                        nc.vector.tensor_copy(sums_sb[:], sums[:])
                        rec = att.tile([1, 512], F32, tag="rec", bufs=2)
                        scr = att.tile([1, 512], F32, tag="scr", bufs=2)
                        nc.vector.reciprocal_approx_accurate(rec[:],
                                                             sums_sb[:],
                                                             scr[:])
                        rec_r = att.tile([1, 512], F32R, tag="rec_r",
                                         bufs=2)
                        nc.vector.tensor_scalar_mul(rec_r[:], rec[:],
                                                    vscale_11)
                        rb = psum.tile([128, 512], F32, tag="psR", bufs=1,
                                       name=f"rb{j}_{qc}")
                        nc.tensor.matmul(rb[:], ones_row_r[:], rec_r[:],
                                         start=True, stop=True)
                        rb_sb = att.tile([128, 512], F32, tag="rb_sb", bufs=2)
                        nc.scalar.activation(rb_sb[:], rb[:], ACTF.Copy)
                        nc.vector.tensor_tensor(attnT[j][:, qsl], aps[:],
                                                rb_sb[:], ALU.mult)
                        mx = att.tile([128, 1], F32, tag="mx", bufs=2)
                        nc.vector.tensor_reduce(mx[:], attnT[j][:, qsl],
                                                axis=mybir.AxisListType.X,
                                                op=ALU.max,
                                                apply_absolute_value=True)
                        nc.vector.tensor_tensor(amax_acc[:], amax_acc[:],
                                                mx[:], ALU.max)

            # ---------------- global amax collective ----------------
            amax_red = cst.tile([128, 1], F32, tag="amax_red")
            nc.gpsimd.partition_all_reduce(amax_red[:], amax_acc[:],
                                           channels=128,
                                           reduce_op=bass_isa.ReduceOp.max)
            pad = cst.tile([1, 8], F32, tag="pad")
            nc.vector.memset(pad[:], 0.0)
            nc.vector.tensor_copy(pad[0:1, 0:1], amax_red[0:1, 0:1])
            cc_in = dram.tile([1, 8], F32, name="cc_in", tag="cc_in")
            cc_out = dram.tile([1, 8], F32, name="cc_out", tag="cc_out",
                               addr_space="Shared")
            nc.sync.dma_start(cc_in[:], pad[:])
            nc.gpsimd.collective_compute(
                "AllReduce", ALU.max,
                replica_groups=[list(range(NCORES))],
                ins=[cc_in.opt()], outs=[cc_out.opt()],
            )
            gmax_row = cst.tile([1, 8], F32, tag="gmax_row")
            nc.sync.dma_start(gmax_row[:], cc_out[:])
            gmax = cst.tile([128, 8], F32, tag="gmax")
            nc.gpsimd.partition_broadcast(gmax[:], gmax_row[:], channels=128)
            sa = cst.tile([128, 1], F32, tag="sa")
            nc.vector.tensor_scalar(out=sa[:], in0=gmax[:, 0:1],
                                    scalar1=1.0 / QMAX, scalar2=1e-8,
                                    op0=ALU.mult, op1=ALU.max)
            inv_sa = cst.tile([128, 1], F32, tag="inv_sa")
            nc.vector.reciprocal(inv_sa[:], sa[:])
            osc = cst.tile([128, 1], F32, tag="osc")
            nc.vector.tensor_tensor(osc[:], sa[:], swo, ALU.mult)

            # ---------------- attn quantization ----------------
            aq = [aqp.tile([128, S], BF16, name=f"aq{j}", tag=f"aq{j}")
                  for j in range(QH_LOC)]
            with tc.tile_pool(name="qtz", bufs=1) as qtz:
                for tcq in range(NTC):
                    tql = slice(512 * tcq, 512 * (tcq + 1))
                    for j in range(QH_LOC):
                        t = qtz.tile([128, 512], F32, tag="aqt", bufs=3)
                        nc.scalar.activation(t[:], attnT[j][:, tql], ACTF.Copy,
                                             bias=MAGIC, scale=inv_sa[:, 0:1])
                        nc.vector.tensor_scalar_add(aq[j][:, tql], t[:],
                                                    -MAGIC)

        # ============ phase 3: o_proj =======================================
        if True:
            with tc.tile_pool(name="opj", bufs=1) as opj:
                for tb in range(NTB):
                    for hc in range(H // 512):
                        ops = psum.tile([128, 512], F32, tag="psA", bufs=3,
                                        name=f"o{tb}_{hc}")
                        for dj in range(DQ_LOC // 128):
                            nc.tensor.matmul(
                                ops[:], aq[dj][:, 128 * tb:128 * (tb + 1)],
                                wo_q[dj][:, 512 * hc:512 * (hc + 1)],
                                start=(dj == 0),
                                stop=(dj == DQ_LOC // 128 - 1))
                        og = opj.tile([128, 512], F32, tag="og", bufs=3)
                        nc.scalar.activation(og[:], ops[:], ACTF.Copy,
                                             scale=osc[:, 0:1])
                        nc.sync.dma_start(
                            out[128 * tb:128 * (tb + 1),
                                512 * hc:512 * (hc + 1)],
                            og[:])


def _build():
    nc = bacc.Bacc("TRN2", target_bir_lowering=False, debug=False,
                   num_devices=NCORES)
    xT = nc.dram_tensor("xT", [H, S], F32, kind="ExternalInput")
    wqT = nc.dram_tensor("wqT", [H, DQ_LOC], F32, kind="ExternalInput")
    wkT = nc.dram_tensor("wkT", [H, DKV_LOC], F32, kind="ExternalInput")
    wvT = nc.dram_tensor("wvT", [H, DKV_LOC], F32, kind="ExternalInput")
    woT = nc.dram_tensor("woT", [DQ_LOC, H], F32, kind="ExternalInput")
    pos = nc.dram_tensor("pos", [1, S], F32, kind="ExternalInput")
    scales = nc.dram_tensor("scales", [1, 16], F32, kind="ExternalInput")
    rt = nc.dram_tensor("rt", [HD, HD], F32, kind="ExternalInput")
    invf = nc.dram_tensor("invf", [128, 1], F32, kind="ExternalInput")
    out = nc.dram_tensor("out", [S, H], F32, kind="ExternalOutput")

    with tile.TileContext(nc) as tc:
        _emit(nc, tc, xT[:], wqT[:], wkT[:], wvT[:], woT[:], pos[:],
              scales[:], rt[:], invf[:], out[:])
    nc.compile()
    return nc


_CACHED = {}
_RUN_KWARGS = {}   # test harness can set {"trace": True, ...}
_LAST = {}         # last BassKernelResults (for profiling in test harness)


def _get_nc():
    if "nc" not in _CACHED:
        _CACHED["nc"] = _build()
    return _CACHED["nc"]


def _host_scales(hidden_states, wq, wk, wv, wo):
    def fq_scale(t):
        return max(float(np.abs(t).max()) / QMAX, 1e-8)

    sx = fq_scale(hidden_states)
    swq = fq_scale(wq)
    swk = fq_scale(wk)
    swv = fq_scale(wv)
    swo = fq_scale(wo)
    s = np.zeros((1, 16), np.float32)
    s[0, 0] = 1.0 / sx
    s[0, 1] = 1.0 / swq
    s[0, 2] = 1.0 / swk
    s[0, 3] = 1.0 / swv
    s[0, 4] = 1.0 / swo
    s[0, 5] = np.float32(sx) * np.float32(swq) / np.float32(np.sqrt(HD))
    s[0, 6] = np.float32(sx) * np.float32(swk)
    s[0, 7] = np.float32(sx) * np.float32(swv)
    s[0, 8] = swo
    return s


def _invfreq():
    # match reference: inv_freq = 1/(theta ** (arange(0,HD,2,f32)/HD)), f32 ops
    e = np.arange(0, HD, 2, dtype=np.float32) / np.float32(HD)
    base = np.float32(THETA) ** e.astype(np.float32)
    invf = (np.float32(1.0) / base.astype(np.float32)).astype(np.float32)
    full = np.concatenate([invf, invf])  # emb = concat([freqs, freqs])
    return np.ascontiguousarray(full.reshape(HD, 1))


def _rot_matrix_T():
    rtm = np.zeros((HD, HD), np.float32)
    half = HD // 2
    idx = np.arange(half)
    rtm[idx, idx + half] = 1.0   # rot[m] = -q[m+64] for m < 64
    rtm[idx + half, idx] = -1.0  # rot[m] = +q[m-64] for m >= 64
    return rtm


def kernel(hidden_states, wq, wk, wv, wo, position_ids):
    hidden_states = np.asarray(hidden_states, dtype=np.float32)
    wq = np.asarray(wq, dtype=np.float32)
    wk = np.asarray(wk, dtype=np.float32)
    wv = np.asarray(wv, dtype=np.float32)
    wo = np.asarray(wo, dtype=np.float32)
    position_ids = np.asarray(position_ids)

    scales = _host_scales(hidden_states, wq, wk, wv, wo)
    invf = _invfreq()
    rtm = _rot_matrix_T()

    in_maps = []
    for c in range(NCORES):
        b, g = c // TP, c % TP
        qsl = slice(DQ_LOC * g, DQ_LOC * (g + 1))
        ksl = slice(DKV_LOC * g, DKV_LOC * (g + 1))
        in_maps.append({
            "xT": np.ascontiguousarray(hidden_states[b].T),
            "wqT": np.ascontiguousarray(wq[qsl, :].T),
            "wkT": np.ascontiguousarray(wk[ksl, :].T),
            "wvT": np.ascontiguousarray(wv[ksl, :].T),
            "woT": np.ascontiguousarray(wo[:, qsl].T),
            "pos": position_ids[b].astype(np.float32).reshape(1, S),
            "scales": scales,
            "rt": rtm,
            "invf": invf,
        })

    nc = _get_nc()
    res_obj = run_bass_kernel_spmd(nc, in_maps, list(range(NCORES)),
                                   **_RUN_KWARGS)
    _LAST["res"] = res_obj
    res = res_obj.results

    outp = np.zeros((B, S, H), np.float64)
    for c in range(NCORES):
        outp[c // TP] += res[c]["out"].astype(np.float64)
    return outp.astype(np.float32)


if __name__ == "__main__":
    rng = np.random.default_rng(0)
    ins = {
        "hidden_states": rng.standard_normal((B, S, H)).astype(np.float32),
        "wq": (rng.standard_normal((NH * HD, H)) * 0.02).astype(np.float32),
        "wk": (rng.standard_normal((NKV * HD, H)) * 0.02).astype(np.float32),
        "wv": (rng.standard_normal((NKV * HD, H)) * 0.02).astype(np.float32),
        "wo": (rng.standard_normal((H, NH * HD)) * 0.02).astype(np.float32),
        "position_ids": np.broadcast_to(np.arange(S), (B, S)).astype(np.int64),
    }
    o = kernel(**ins)
    print("out", o.shape, o.dtype, float(np.abs(o).max()))
